# revision 28
# baseline (speedup 1.0000x reference)
"""Trainium2 kernel for nn_ConnectedThresholdLayer (gated connected-filter on
morphological max-trees + pixel reconstruction).

Mathematical reformulation (exactly equivalent to the reference on valid
trees, which setup_inputs always produces):

  The reference computes, per (b,c) tree, S[n] = sum of s[k] over the
  root->n path (pointer-doubling with K=12 covers depth < 4096; actual
  random-recursive-tree depth is ~35), with
      s[k] = gate[k] * (level[k] - level[parent[k]]),  s[root] = level[root]
      gate[k] = (sigmoid(a_scaled - thr_norm) >= 0.5)  ==  (attr[k] >= thr)
  (min-max scaling is strictly monotone, so the 0.5-sigmoid threshold
  reduces exactly to the raw comparison), then out[pix] = S[node[pix]].

  Path sums over a tree are an Euler-tour prefix scan: entering node k adds
  s[k], leaving subtracts it; the running sum at k's entry event equals
  S[k].  The host derives the tour layout from the int32 `parent` tensor
  alone; the device does all FP arithmetic, fully dense -- no
  data-dependent addressing on device.

  Byte-minimal event encoding: with lam[j] = level of the node the tour is
  AT after event j, every event's contribution is
      gate[j] * (lam[j] - lam[j-1])
  (enter n: lam jumps lv[par]->lv[n] = +res; exit n: lv[n]->lv[par] =
  -res).  lam travels once as bf16 (plus one duplicated boundary column
  per partition row so the shifted access never crosses rows).  The gate
  travels as a ONE-BYTE monotone code: uint8 c = clip(((bits(attr)+d)>>21)
  - base) where d aligns thr onto a 2^21 bit boundary -- for non-negative
  f32, bit patterns order like values, so (c >= code_thr) == (attr >= thr)
  EXACTLY, at 1/4 the bytes of an f32 attr stream.

  Device pipeline per tree (engines split so DMA/DVE/ACT all overlap):
    ACT : M = Identity(512*c - 512*(code_thr-0.5))   # >=+256 if gate else <=-256
    DVE : A = min(lam[j],  M[j])    # gate: lam_j,  else M_j
    DVE : B = min(lam[j-1],M[j])    # gate: lam_j-1, else M_j (same value!)
    DVE : sr = scan: state = (A_j + state) - B_j     # fp32 running sum
          (gate=0 events contribute A-B = M-M = 0 exactly; gate=1
           contribute lam_j - lam_j-1; |M|<=2^17 so no f32 absorption)
    PE  : carry = strict_upper_ones.T @ sr[:, -1]    # cross-partition carry
    ACT : R = Identity(sr + carry)  -> bf16          # same ACT table: no
                                                    # table-load thrash
  The host gathers R at each pixel's entry-event position (pure copies).

Sharding: trees are independent per (b,c); the 24 trees go 3-per-NeuronCore
across 8 cores (data parallel, zero cross-device communication).

Host does ONLY integer index planning (from `parent` / `pixel_to_node`) and
bit/data marshaling (event reordering, uint8 threshold codes, bf16 casts,
inverse map on the returned scan); every floating-point operation on
attr/level/thr values runs on the NeuronCores.
"""

import ml_dtypes
import numpy as np

P = 128            # SBUF partitions
TREES_PER_CORE = 3
N_CORES = 8
BF16 = ml_dtypes.bfloat16

BUFS = 2           # tile-pool depth (pipeline depth across trees)

_CACHE = {}


def _thr_code_params(thr_f):
    """uint8 monotone code of non-negative f32 at 2^21-bit granularity with
    a code boundary exactly at thr.  Returns (delta, base, code_thr) or None
    if thr is too small for the construction."""
    tb = int(np.float32(thr_f).view(np.uint32))
    v_bits = -(-tb // (1 << 21)) << 21       # ceil to 2^21 boundary
    delta = v_bits - tb
    v21 = v_bits >> 21
    if v21 < 1:
        return None
    base = max(v21 - 128, 0)
    code_thr = v21 - base                    # in [1, 255+]; clip guard below
    if code_thr > 255:
        return None
    return delta, base, code_thr


def _attr_codes(a_f32, delta, base):
    """uint8 codes: clip(((bits(a)+delta)>>21) - base, 0, 255).  Monotone in
    a for a >= 0, with code >= code_thr  <=>  a >= thr  exactly."""
    b = a_f32.astype(np.float32).view(np.uint32).astype(np.int64)
    return np.clip(((b + delta) >> 21) - base, 0, 255).astype(np.uint8)


# ----------------------------------------------------------------------------
# Host-side integer planning (uses only `parent` / `pixel_to_node`)
# ----------------------------------------------------------------------------

def _tree_plan(parent):
    """parent: (N,) int with parent[n] < n for n >= 1.

    Returns ev_enter (N,) int64: position of each node's entry event in the
    2N-long Euler event stream.  Root (node 0) is excluded from the stream;
    positions 0 and 2N-1 are zero-contribution pads, and ev_enter[0] = 0
    (the running sum there is 0; the root's base level is added globally).
    """
    N = parent.shape[0]
    par = parent.astype(np.int64)

    # depth (= #edges to root) via pointer doubling with absorbing root
    val = (np.arange(N) != 0).astype(np.int64)
    a = par.copy()
    a[0] = 0
    for _ in range(20):
        if not a.any():
            break
        val = val + val[a]
        a = a[a]
    depth = val
    maxd = int(depth.max())
    if maxd >= 4096:
        return None, None, maxd

    # subtree sizes, bottom-up by depth level
    size = np.ones(N, np.int64)
    order = np.argsort(depth, kind="stable")
    bounds = np.searchsorted(depth[order], np.arange(maxd + 2))
    for d in range(maxd, 0, -1):
        nodes = order[bounds[d]:bounds[d + 1]]
        if len(nodes) == 0:
            continue
        size += np.bincount(par[nodes], weights=size[nodes],
                            minlength=N).astype(np.int64)

    # prefix of earlier-sibling subtree sizes (children visited in index order)
    sibord = np.argsort(par[1:], kind="stable") + 1
    sz = size[sibord]
    cs = np.cumsum(sz) - sz
    pgroup = par[sibord]
    first = np.ones(len(sibord), bool)
    first[1:] = pgroup[1:] != pgroup[:-1]
    base = np.where(first, cs, 0)
    np.maximum.accumulate(base, out=base)
    bss = np.zeros(N, np.int64)
    bss[sibord] = cs - base

    # preorder index = path-sum of (1 + bss) excluding root, via doubling
    c = 1 + bss
    c[0] = 0
    S = c
    a = par.copy()
    a[0] = 0
    for _ in range(20):
        if not a.any():
            break
        S = S + S[a]
        a = a[a]
    pre = S
    ev_enter = 2 * pre - depth
    ev_enter[0] = 0
    return ev_enter, size, maxd


def _host_preprocess(attr, level, thr, parent, pixel_to_node):
    """Returns (in_maps for 8 cores, q (T, HW) int32 event positions, F)."""
    B, C, N = attr.shape
    T = B * C
    twoN = 2 * N
    F = twoN // P
    attr2 = np.ascontiguousarray(attr.reshape(T, N))
    level2 = np.ascontiguousarray(level.reshape(T, N))
    par2 = np.ascontiguousarray(parent.reshape(T, N))
    pix2 = pixel_to_node.reshape(T, -1)

    thr_f = np.float32(np.asarray(thr).reshape(-1)[0])
    cp = _thr_code_params(thr_f)
    if cp is None:
        return None, None, None
    delta, cbase, code_thr = cp

    # lam[j] = level of the node the tour is AT after event j; the device
    # reconstructs every event contribution as gate * (lam[j] - lam[j-1]).
    evattr = np.empty((T, twoN), np.float32)
    evlam = np.zeros((T, twoN), np.float32)
    q = np.empty((T, pix2.shape[1]), np.int32)
    nr = np.arange(1, N)
    for t in range(T):
        ev_enter, size, maxd = _tree_plan(par2[t])
        if maxd >= 4096:
            # reference's K=12 pointer doubling truncates paths longer than
            # 4096; the Euler scan computes the untruncated sum -> not
            # equivalent. Caller must use the exact fallback.
            return None, None, None
        ev_exit = ev_enter + 2 * size - 1
        at, lv, pr = attr2[t], level2[t], par2[t]
        en = ev_enter[nr]
        ex = ev_exit[nr]
        plv = lv[pr[nr]]
        # event 0 carries the root base level: attr=+huge forces gate=1 and
        # lam jumps 0 -> rootlv, so the scan starts at the root level and no
        # separate per-tree parameter add is needed on device.  Position
        # 2N-1 is after every entry event, so its contribution is never
        # read.
        evattr[t, 0] = 3.0e38
        evlam[t, 0] = lv[0]
        evattr[t, twoN - 1] = at[0]
        evattr[t, en] = at[nr]
        evlam[t, en] = lv[nr]
        evattr[t, ex] = at[nr]
        evlam[t, ex] = plv         # back at the parent => exact negation
        q[t] = ev_enter[np.clip(pix2[t], 0, N - 1)].astype(np.int32)

    # lam travels with one extra leading column per partition row (the
    # previous row's last lam; 0 for row 0) so the shifted access
    # A=lam[:,1:], B=lam[:,:-1] never crosses partitions.  FL8 pads the
    # per-tree block to a 16B multiple.
    FL8 = F + 8
    in_maps = []
    for c in range(N_CORES):
        a8 = np.zeros((P, TREES_PER_CORE * F), np.uint8)
        lam = np.zeros((P, TREES_PER_CORE * FL8), BF16)
        for k in range(TREES_PER_CORE):
            t = c * TREES_PER_CORE + k
            a8[:, k * F:(k + 1) * F] = _attr_codes(
                evattr[t], delta, cbase).reshape(P, F)
            flat = np.concatenate(
                [np.zeros(1, np.float32), evlam[t]]).astype(BF16)
            lam[:, k * FL8:k * FL8 + F + 1] = (
                np.lib.stride_tricks.sliding_window_view(flat, F + 1)[::F])
        in_maps.append({"a8": a8, "lam": lam})
    return in_maps, q, F


# ----------------------------------------------------------------------------
# Device program
# ----------------------------------------------------------------------------

def _build_nc(F, repeat=1, thr=500.0, bufs=None, code_thr=None):
    import concourse.bacc as bacc
    import concourse.mybir as mybir
    import concourse.tile as tile

    f32 = mybir.dt.float32
    bf16 = mybir.dt.bfloat16
    u8 = mybir.dt.uint8
    op = mybir.AluOpType
    act = mybir.ActivationFunctionType
    if bufs is None:
        bufs = BUFS
    if code_thr is None:
        cp = _thr_code_params(np.float32(thr))
        code_thr = cp[2]

    from concourse.masks import make_upper_triangular

    FL8 = F + 8
    TC = TREES_PER_CORE
    # gate mask: M = 512*code - 512*(code_thr - 0.5).  Codes are integers,
    # so M >= +256 when code >= code_thr, else M <= -256; all values are
    # multiples of 256 with <= 9 significant bits => exact in bf16.
    MSCALE = 512.0
    MBIAS = float(-512.0 * (code_thr - 0.5))

    nc = bacc.Bacc("TRN2", target_bir_lowering=False, debug=False,
                   num_devices=N_CORES)
    a8 = nc.dram_tensor("a8", [P, TC * F], u8, kind="ExternalInput")
    lam = nc.dram_tensor("lam", [P, TC * FL8], bf16, kind="ExternalInput")
    Rout = nc.dram_tensor("R", [P, TC * F], bf16, kind="ExternalOutput")

    with tile.TileContext(nc) as tc:
        with tc.tile_pool(name="consts", bufs=1) as cpool, \
                tc.tile_pool(name="sbuf", bufs=bufs) as pool, \
                tc.psum_pool(name="psum", bufs=2) as ppool:
            # strict-upper ones: U.T @ rowtotals = exclusive prefix sum over
            # partitions (the cross-partition scan carry) in one PE matmul
            U = cpool.tile([P, P], f32, tag="U")
            make_upper_triangular(nc, U[:], val=1.0, diag=False)
            # M-pass bias as an SBUF vector (float biases for non-Copy
            # activations need a pre-registered const AP)
            mb = cpool.tile([P, 1], f32, tag="mbias")
            nc.vector.memset(mb[:], MBIAS)

            for t in [tt % TC for tt in range(TC * repeat)]:
                ac = slice(t * F, (t + 1) * F)
                lc = slice(t * FL8, t * FL8 + F + 1)
                a8t = pool.tile([P, F], u8, tag="a8")
                nc.sync.dma_start(a8t, a8.ap()[:, ac])
                lamt = pool.tile([P, F + 1], bf16, tag="lam")
                nc.sync.dma_start(lamt, lam.ap()[:, lc])

                # gate mask on the otherwise idle scalar engine (Identity
                # only -- no activation-table thrash)
                M = pool.tile([P, F], bf16, tag="M")
                nc.scalar.activation(out=M[:], in_=a8t[:], func=act.Identity,
                                     scale=MSCALE, bias=mb[:, 0:1])

                # A-B = gate*(lam_j - lam_j-1): 0 for gated-off events since
                # both mins then pick the SAME M value
                A = pool.tile([P, F], bf16, tag="A")
                nc.vector.tensor_tensor(out=A[:], in0=lamt[:, 1:F + 1],
                                        in1=M[:], op=op.min)
                Bt = pool.tile([P, F], bf16, tag="B")
                nc.vector.tensor_tensor(out=Bt[:], in0=lamt[:, 0:F],
                                        in1=M[:], op=op.min)

                # per-partition running sum in fp32; last column = row total
                sr = pool.tile([P, F], f32, tag="sr")
                nc.vector.tensor_tensor_scan(
                    out=sr[:], data0=A[:], data1=Bt[:], initial=0.0,
                    op0=op.add, op1=op.subtract)

                # cross-partition carry on the otherwise idle PE
                carry = ppool.tile([P, 1], f32, tag="carry")
                nc.tensor.matmul(carry[:], U[:], sr[:, F - 1:F],
                                 start=True, stop=True)
                csb = pool.tile([P, 1], f32, tag="csb")
                nc.vector.tensor_scalar_add(csb[:], carry[:], 0.0)

                # seed each partition with its carry (ACT bias is a
                # per-partition vector); single bf16 downcast at the end
                rf = pool.tile([P, F], bf16, tag="rf")
                nc.scalar.activation(out=rf[:], in_=sr[:], func=act.Identity,
                                     scale=1.0, bias=csb[:, 0:1])
                nc.sync.dma_start(Rout.ap()[:, ac], rf[:])
    nc.compile()
    return nc


def _get_nc(F, thr):
    key = ("nc", F, float(thr), BUFS)
    if key not in _CACHE:
        _CACHE[key] = _build_nc(F, thr=thr)
    return _CACHE[key]


# ----------------------------------------------------------------------------
# Fallback: exact f32 emulation of the reference (invalid trees, deep trees,
# degenerate thresholds)
# ----------------------------------------------------------------------------

def _fallback_reference(attr, level, thr, parent, pixel_to_node):
    B, C, N = attr.shape
    # replicate reference's scaled-sigmoid gate semantics
    amin = attr.min(-1, keepdims=True)
    amax = attr.max(-1, keepdims=True)
    denom = np.maximum(amax - amin, np.float32(1e-6))
    a_s = ((attr - amin) / denom).astype(np.float32)
    t_n = ((np.float32(thr.reshape(-1)[0]) - amin) / denom).astype(np.float32)
    d = (a_s - t_n).astype(np.float32)
    soft = (1.0 / (1.0 + np.exp(-d.astype(np.float64)))).astype(np.float32)
    gate = (soft >= 0.5).astype(np.float32)
    pixel_to_node = np.clip(pixel_to_node, 0, N - 1)
    pl = np.take_along_axis(level, np.clip(parent, 0, N - 1).astype(np.int64),
                            axis=-1)
    s = gate * (level - pl)
    s[..., 0] = level[..., 0]
    s = np.concatenate([s, np.zeros((B, C, 1), np.float32)], axis=-1)
    p = np.concatenate([np.clip(parent, 0, N).astype(np.int32),
                        np.full((B, C, 1), N, np.int32)], axis=-1)
    p[..., 0] = N
    S = s.astype(np.float32)
    pp = p.astype(np.int64)
    for _ in range(12):
        S = (S + np.take_along_axis(S, pp, axis=-1)).astype(np.float32)
        pp = np.take_along_axis(pp, pp, axis=-1)
    S = S[..., :N]
    out = np.take_along_axis(S, pixel_to_node.astype(np.int64), axis=-1)
    HW = pixel_to_node.shape[-1]
    H = int(np.sqrt(HW))
    return out.reshape(B, C, H, HW // H).astype(np.float32)


# ----------------------------------------------------------------------------
# Entry point
# ----------------------------------------------------------------------------

def kernel(attr, level, thr_raw, parent, pixel_to_node):
    attr = np.asarray(attr, np.float32)
    level = np.asarray(level, np.float32)
    thr_raw = np.asarray(thr_raw, np.float32)
    parent = np.asarray(parent)
    pixel_to_node = np.asarray(pixel_to_node)
    B, C, N = attr.shape
    HW = pixel_to_node.shape[-1]
    H = int(np.sqrt(HW))

    par2 = parent.reshape(-1, N)
    valid = bool(np.all(par2[:, 1:] < np.arange(1, N)) and np.all(par2 >= 0))
    thr_f = np.float32(thr_raw.reshape(-1)[0])
    # the uint8 gate codes are exact only for finite positive thr and
    # non-negative attr; otherwise take the exact host path.
    if (not valid or B * C != N_CORES * TREES_PER_CORE or (2 * N) % P != 0
            or not np.isfinite(thr_f) or not (thr_f > 1e-30)
            or not bool(np.all(attr >= 0))
            or _thr_code_params(thr_f) is None):
        return _fallback_reference(attr, level, thr_raw, parent, pixel_to_node)

    in_maps, q, F = _host_preprocess(attr, level, thr_raw, parent,
                                     pixel_to_node)
    if in_maps is None:  # depth >= 4096: doubling truncation applies
        return _fallback_reference(attr, level, thr_raw, parent,
                                   pixel_to_node)
    try:
        nc = _get_nc(F, thr_f)
        from concourse.bass_utils import run_bass_kernel_spmd
        res = run_bass_kernel_spmd(nc, in_maps, core_ids=list(range(N_CORES)))
    except Exception as e:  # infra failure: still return a correct result
        import traceback
        traceback.print_exc()
        print(f"kernel: device path failed ({type(e).__name__}); "
              "falling back to host emulation")
        return _fallback_reference(attr, level, thr_raw, parent,
                                   pixel_to_node)

    out = np.empty((B * C, HW), np.float32)
    for c in range(N_CORES):
        R = res.results[c]["R"].view(BF16).reshape(P, TREES_PER_CORE, F)
        for k in range(TREES_PER_CORE):
            t = c * TREES_PER_CORE + k
            out[t] = np.ascontiguousarray(R[:, k, :]).ravel()[q[t]].astype(
                np.float32)
    return out.reshape(B, C, H, HW // H)
